# revision 8
# baseline (speedup 1.0000x reference)
"""FastVAR cross-attention block kernel for 8 Trainium2 NeuronCores.

Sharding: 2 batches x 4 head-groups (4 heads each) = 8 cores.
Per-core device program (identical SPMD program, per-core data):
  phase 1: qkv projection (bf16 matmul, fp32 psum) -> l2-norms (DVE) ->
    normalize q (with per-head scale folded in) and k -> RoPE on a
    de-interleaved [even|odd] feature layout (q on DVE, k on GPSIMD) ->
    PE transposes to feature-major qT/kT.
  phase 2: per q-chunk (512,512,512 + merged 128 tail), alternating the
    two head-pairs so ACT (exp) never idles: scores for both heads of a
    pair into one psum tile -> single merged exp (scale=1, the per-head
    scale is folded into q) -> AV with ones-augmented V (denominator for
    free).  Normalization (reciprocal + broadcast via DRAM round-trip)
    happens off the critical path, overlapped with the next q-chunk.
  phase 3: projection (partial over this core's 256 head-channels),
    bf16 output.
Host: top-k token selection (replicates reference argsort bitwise on CPU
jax), gather, weight slicing/permutation, partial-sum reduction,
scatter + residual.
"""

import math
import os
import sys
from contextlib import ExitStack

import numpy as np

import concourse.bass as bass
import concourse.bacc as bacc
import concourse.tile as tile
from concourse.tile import add_dep_helper
from concourse import mybir
from concourse import bass_utils

# ---------------------------------------------------------------- constants
B = 2
L = 4096
C = 1024
NH = 16
DH = 64
NREM = 1638          # num_remain for this problem
NT = 1664            # padded token count (13 * 128)
HPG = 4              # heads per core (16 heads / 4 groups)
N_CORES = 8

F32 = mybir.dt.float32
BF16 = mybir.dt.bfloat16

# q-chunks: three full 512-wide chunks + a 128-wide tail that reuses the
# same (padded) psum tiles so each head's accumulation group stays in its
# own PSUM bank.
QCHUNKS = [(0, 512), (512, 512), (1024, 512), (1536, 128)]


class Cfg:
    def __init__(self, NT, NTR, C, HPG, DH, has_bias=False):
        self.NT, self.NTR, self.C, self.HPG, self.DH = NT, NTR, C, HPG, DH
        self.has_bias = has_bias
        self.NC = NT // 128          # token chunks
        self.CH = C // 128           # contraction chunks
        self.QK = 2 * HPG * DH       # q+k feature width (512)
        self.F = 3 * HPG * DH        # qkv feature width (768)
        self.HC = HPG * DH           # head channels per core (256)
        self.HCC = self.HC // 128    # proj contraction chunks (2)


FULL_CFG = Cfg(NT=NT, NTR=NREM, C=C, HPG=HPG, DH=DH)


# ---------------------------------------------------------------- device IR
def emit_core_program(tc, outs, ins, cfg):
    """Emit the per-core Tile program. ins/outs are dicts of DRAM APs."""
    nc = tc.nc
    NTc, NC, Cc, CH = cfg.NT, cfg.NC, cfg.C, cfg.CH
    QK, F = cfg.QK, cfg.F
    X = mybir.AxisListType.X

    xmT, wqkvT = ins["xmT"], ins["wqkvT"]
    ropeC, ropeS = ins["ropeC"], ins["ropeS"]
    wpT, scvec = ins["wpT"], ins["scvec"]
    outp = outs["outp"]

    with ExitStack() as ctx:
        const = ctx.enter_context(tc.tile_pool(name="const", bufs=1))

        # ---- resident inputs (few, large DMAs; weights first) ----------
        w = const.tile([128, CH, F], BF16, tag="w")
        nc.sync.dma_start(w[:], wqkvT.rearrange("(ci p) f -> p ci f", p=128))
        xmB = const.tile([128, NC, CH, 128], BF16, tag="xmB")
        xmT_r = xmT.rearrange("(ci p) nt -> p ci nt", p=128)
        for t in range(NC):
            nc.sync.dma_start(xmB[:, t, :, :],
                              xmT_r[:, :, t * 128:(t + 1) * 128])
        ropeCt = const.tile([128, NC, DH], BF16, tag="ropeCt")
        nc.gpsimd.dma_start(ropeCt[:], ropeC.rearrange("(t p) d -> p t d", p=128))
        ropeSt = const.tile([128, NC, DH], BF16, tag="ropeSt")
        nc.gpsimd.dma_start(ropeSt[:], ropeS.rearrange("(t p) d -> p t d", p=128))
        wp = const.tile([128, cfg.HCC, Cc], BF16, tag="wp")
        nc.gpsimd.dma_start(wp[:], wpT.rearrange("(hc p) c -> p hc c", p=128))
        scv = const.tile([128, 2 * HPG], F32, tag="scv")
        nc.gpsimd.dma_start(scv[:], scvec[0:1, :].to_broadcast((128, 2 * HPG)))
        ident = const.tile([128, 128], BF16, tag="ident")
        nc.gpsimd.dma_start(ident[:], ins["ident"][:])
        if cfg.has_bias:
            ones_row = const.tile([1, NTc], BF16, tag="ones_row")
            nc.sync.dma_start(ones_row[:], xmT[Cc:Cc + 1, :])
            w_bias = const.tile([1, F], BF16, tag="wb")
            nc.sync.dma_start(w_bias[:], wqkvT[Cc:Cc + 1, :])

        # feature-major q/k: [128, {qp0,qp1,kp0,kp1}, t, tok]
        qkT = const.tile([128, 4, NC, 128], BF16, tag="qkT")
        vav = const.tile([128, NC, HPG, DH + 1], BF16, tag="vav")
        oPair = [const.tile([128, NTc], BF16, name=f"oP{i}", tag=f"oP{i}")
                 for i in range(cfg.HCC)]

        wk = ctx.enter_context(tc.tile_pool(name="wk", bufs=3))

        # ---------------- phase 1: qkv + norm + rope + transposes ----------
        with tc.tile_pool(name="p1ps", bufs=2, space="PSUM") as p1, \
             tc.tile_pool(name="tpps", bufs=2, space="PSUM") as tp:
            for t in range(NC):
                ps = p1.tile([128, F], F32)
                n_ci = CH + 1 if cfg.has_bias else CH
                for ci in range(n_ci):
                    lhs = (xmB[:, t, ci, :] if ci < CH
                           else ones_row[:, t * 128:(t + 1) * 128])
                    for n0 in range(0, F, 512):
                        nn = min(512, F - n0)
                        rhsw = (w[:, ci, n0:n0 + nn] if ci < CH
                                else w_bias[:, n0:n0 + nn])
                        nc.tensor.matmul(
                            ps[:, n0:n0 + nn], lhs, rhsw,
                            start=(ci == 0), stop=(ci == n_ci - 1),
                        )
                qkv = wk.tile([128, F], BF16, tag="qkv")
                nc.scalar.copy(qkv[:], ps[:])

                # l2 norms of q,k along dh (free-dim reduce on DVE)
                sq = wk.tile([128, QK], BF16, tag="sq")
                nc.vector.tensor_mul(sq[:], qkv[:, 0:QK], qkv[:, 0:QK])
                ss = wk.tile([128, 8, 1], F32, tag="ss")
                nc.vector.reduce_sum(
                    ss.rearrange("p h one -> p (h one)"),
                    sq.rearrange("p (h d) -> p h d", d=DH), axis=X)
                sroot = wk.tile([128, 8], F32, tag="sroot")
                nc.scalar.activation(
                    sroot[:], ss.rearrange("p h one -> p (h one)"),
                    mybir.ActivationFunctionType.Sqrt)
                nc.vector.tensor_scalar_max(sroot[:], sroot[:], 1e-12)
                rr = wk.tile([128, 8], F32, tag="rr")
                nc.vector.reciprocal(rr[:], sroot[:])
                # fold per-head attention scale into q's normalizer
                rr2 = wk.tile([128, 8, 1], BF16, tag="rr2")
                nc.vector.tensor_mul(
                    rr2.rearrange("p h one -> p (h one)"), rr[:], scv[:])
                qkn = wk.tile([128, QK], BF16, tag="qkn")
                nc.vector.tensor_mul(
                    qkn.rearrange("p (h d) -> p h d", d=DH),
                    qkv[:, 0:QK].rearrange("p (h d) -> p h d", d=DH),
                    rr2.to_broadcast((128, 8, DH)))

                # rope on de-interleaved [even(32)|odd(32)] feature layout:
                # out_e = c*t_e - s*t_o ; out_o = s*t_e + c*t_o
                # q half on DVE, k half on GPSIMD
                rctb = ropeCt[:, t:t + 1, :].to_broadcast((128, HPG, DH))
                rstb = ropeSt[:, t:t + 1, :].to_broadcast((128, HPG, DH))
                ca = wk.tile([128, QK], BF16, tag="ca")
                sa = wk.tile([128, QK], BF16, tag="sa")
                qkr = wk.tile([128, QK], BF16, tag="qkr")
                HQ = QK // 2   # 256

                def rh(tt, lo):   # [128, (4 heads, 64)] slice starting at lo
                    return tt[:, lo:lo + HQ].rearrange(
                        "p (h d) -> p h d", d=DH)

                def rev(tt, lo):  # even-32 half per head
                    return tt[:, lo:lo + HQ].rearrange(
                        "p (h half d) -> p h half d", half=2, d=32)[:, :, 0:1, :]

                def rod(tt, lo):
                    return tt[:, lo:lo + HQ].rearrange(
                        "p (h half d) -> p h half d", half=2, d=32)[:, :, 1:2, :]

                nc.vector.tensor_mul(rh(ca, 0), rh(qkn, 0), rctb)
                nc.vector.tensor_mul(rh(sa, 0), rh(qkn, 0), rstb)
                nc.vector.tensor_sub(rev(qkr, 0), rev(ca, 0), rod(sa, 0))
                nc.vector.tensor_add(rod(qkr, 0), rev(sa, 0), rod(ca, 0))
                nc.gpsimd.tensor_mul(rh(ca, HQ), rh(qkn, HQ), rctb)
                nc.gpsimd.tensor_mul(rh(sa, HQ), rh(qkn, HQ), rstb)
                nc.gpsimd.tensor_sub(rev(qkr, HQ), rev(ca, HQ), rod(sa, HQ))
                nc.gpsimd.tensor_add(rod(qkr, HQ), rev(sa, HQ), rod(ca, HQ))

                # v (+ softmax-denominator ones column)
                nc.gpsimd.tensor_copy(
                    vav[:, t, :, 0:DH],
                    qkv[:, QK:F].rearrange("p (h d) -> p h d", d=DH))
                pad0 = cfg.NTR - (NC - 1) * 128
                if t == NC - 1 and pad0 < 128:
                    nc.vector.memset(vav[:, t, :, DH:DH + 1], 0.0)
                    nc.vector.memset(vav[0:pad0, t, :, DH:DH + 1], 1.0)
                else:
                    nc.vector.memset(vav[:, t, :, DH:DH + 1], 1.0)

                # feature-major q,k via PE transpose (one batched copy out)
                tps = tp.tile([128, 512], BF16, name="tps", tag="tps")
                for j in range(4):
                    nc.tensor.transpose(
                        tps[:, j * 128:(j + 1) * 128],
                        qkr[:, j * 128:(j + 1) * 128], ident[:])
                dst = qkT[:, :, t, :]
                src = tps.rearrange("p (a b) -> p a b", b=128)
                if t % 2 == 0:
                    nc.scalar.copy(dst, src)
                else:
                    nc.vector.tensor_copy(dst, src)

        # flattened feature-major views for scores
        qT_h = {}
        kT_h = {}
        for pair in range(2):
            for h in range(2):
                psl = slice(64 * h, 64 * (h + 1))
                qT_h[(pair, h)] = qkT[psl, pair, :, :].rearrange(
                    "p a b -> p (a b)")
                kT_h[(pair, h)] = qkT[psl, 2 + pair, :, :]

        # ---------------- phase 2: attention ------------------------------
        pex = ctx.enter_context(tc.tile_pool(name="pex", bufs=3))
        pno = ctx.enter_context(tc.tile_pool(name="pno", bufs=2))
        pd = ctx.enter_context(tc.tile_pool(name="dscr", bufs=2, space="DRAM"))

        def normalize(pair, h, oTs, q0, qn, nq):
            """oPair[pair][64h:64h+64, q0:q0+qn] = oTs[0:64,h,:]/den."""
            den_d = pd.tile([1, qn], BF16, name="den_d", tag="den_d")
            nc.sync.dma_start(den_d[:], oTs[DH:DH + 1, h, :])
            dtok = pno.tile([128, nq], BF16, tag="dtok")
            nc.sync.dma_start(
                dtok[:], den_d.rearrange("one (c p) -> (one p) c", p=128))
            rtok = pno.tile([128, nq], BF16, tag="rtok")
            with nc.allow_low_precision(reason="bf16 softmax denom recip"):
                nc.vector.reciprocal(rtok[:], dtok[:])
            dram_r = pd.tile([nq, 128], BF16, name="dram_r", tag="dram_r")
            nc.sync.dma_start(dram_r.rearrange("c p -> p c"), rtok[:])
            bc = pno.tile([64, qn], BF16, tag="bc")
            bc_src = bass.AP(tensor=dram_r.tensor, offset=dram_r.offset,
                             ap=[[0, 64], [1, qn]])
            nc.sync.dma_start(bc[:], bc_src)
            nc.vector.tensor_mul(
                oPair[pair][64 * h:64 * (h + 1), q0:q0 + qn],
                oTs[0:DH, h, :], bc[:])

        with tc.tile_pool(name="p2ps", bufs=1, space="PSUM") as p2:
            for (q0, qn) in QCHUNKS:
                oTT = {}
                for pair in range(2):
                    # padded to 512 per head so each head's accumulation
                    # group owns a full PSUM bank
                    oTT[pair] = p2.tile([DH + 1, 2, 512], F32,
                                        name=f"oTT{pair}", tag=f"oTT{pair}")
                prev = {0: None, 1: None}
                for kb in range(NC):
                    for pair in range(2):
                        sc = p2.tile([128, 2, 512], F32, name=f"sc{pair}",
                                     tag=f"sc{pair}")
                        for h in range(2):
                            nc.tensor.matmul(
                                sc[:, h, 0:qn],
                                kT_h[(pair, h)][:, kb, :],
                                qT_h[(pair, h)][:, q0:q0 + qn],
                                start=True, stop=True)
                        ex = pex.tile([128, 2, qn], BF16, name=f"ex{pair}",
                                      tag=f"ex{pair}")
                        nc.scalar.activation(
                            ex[:], sc[:, :, 0:qn],
                            mybir.ActivationFunctionType.Exp)
                        if prev[pair] is not None:
                            pex_t, pkb = prev[pair]
                            for h in range(2):
                                nc.tensor.matmul(
                                    oTT[pair][:, h, 0:qn],
                                    vav[:, pkb, 2 * pair + h, :],
                                    pex_t[:, h, :],
                                    start=(pkb == 0), stop=(pkb == NC - 1))
                        prev[pair] = (ex, kb)
                for pair in range(2):
                    pex_t, pkb = prev[pair]
                    for h in range(2):
                        nc.tensor.matmul(
                            oTT[pair][:, h, 0:qn],
                            vav[:, pkb, 2 * pair + h, :],
                            pex_t[:, h, :],
                            start=(pkb == 0), stop=(pkb == NC - 1))
                    oTs = pno.tile([DH + 1, 2, qn], BF16, name=f"oTs{pair}",
                                   tag=f"oTs{pair}")
                    nc.vector.tensor_copy(oTs[:], oTT[pair][:, :, 0:qn])
                    for h in range(2):
                        normalize(pair, h, oTs, q0, qn, max(1, qn // 128))

        # ---------------- phase 3: projection ------------------------------
        po = ctx.enter_context(tc.tile_pool(name="po", bufs=3))
        with tc.tile_pool(name="p3ps", bufs=2, space="PSUM") as p3:
            for t in range(NC):
                tsl = slice(t * 128, (t + 1) * 128)
                ps = p3.tile([128, Cc], F32)
                for hc in range(cfg.HCC):
                    for n0 in range(0, Cc, 512):
                        nn = min(512, Cc - n0)
                        nc.tensor.matmul(
                            ps[:, n0:n0 + nn], oPair[hc][:, tsl],
                            wp[:, hc, n0:n0 + nn],
                            start=(hc == 0), stop=(hc == cfg.HCC - 1))
                ob = po.tile([128, Cc], BF16)
                if t % 2 == 0:
                    nc.scalar.copy(ob[:], ps[:])
                else:
                    nc.vector.tensor_copy(ob[:], ps[:])
                nc.sync.dma_start(outp[tsl, :], ob[:])


# ---------------------------------------------------------------- build
def declare_io(nc, cfg):
    crow = cfg.C + 1 if cfg.has_bias else cfg.C
    ins = {
        "xmT": nc.dram_tensor("xmT", [crow, cfg.NT], BF16,
                              kind="ExternalInput").ap(),
        "wqkvT": nc.dram_tensor("wqkvT", [crow, cfg.F], BF16,
                                kind="ExternalInput").ap(),
        "ropeC": nc.dram_tensor("ropeC", [cfg.NT, cfg.DH], BF16,
                                kind="ExternalInput").ap(),
        "ropeS": nc.dram_tensor("ropeS", [cfg.NT, cfg.DH], BF16,
                                kind="ExternalInput").ap(),
        "wpT": nc.dram_tensor("wpT", [cfg.HC, cfg.C], BF16,
                              kind="ExternalInput").ap(),
        "scvec": nc.dram_tensor("scvec", [1, 2 * cfg.HPG], F32,
                                kind="ExternalInput").ap(),
        "ident": nc.dram_tensor("ident", [128, 128], BF16,
                                kind="ExternalInput").ap(),
    }
    outs = {
        "outp": nc.dram_tensor("outp", [cfg.NT, cfg.C], BF16,
                               kind="ExternalOutput").ap(),
    }
    return ins, outs


_BUILD_CACHE = {}

if os.environ.get("LDW_OPT", "0") == "1":
    _orig_run_command = bass_utils.run_command

    def _patched_run_command(argv, **kw):
        argv = ["--enable-ldw-opt=true" if a == "--enable-ldw-opt=false" else a
                for a in argv]
        return _orig_run_command(argv, **kw)

    bass_utils.run_command = _patched_run_command


def build_full_program(has_bias=False):
    key = ("full", has_bias)
    if key in _BUILD_CACHE:
        return _BUILD_CACHE[key]
    cfg = Cfg(NT=NT, NTR=NREM, C=C, HPG=HPG, DH=DH, has_bias=has_bias)
    nc = bacc.Bacc("TRN2", target_bir_lowering=False, debug=False,
                   num_devices=N_CORES)
    ins, outs = declare_io(nc, cfg)
    with tile.TileContext(nc) as tc:
        emit_core_program(tc, outs, ins, cfg)
    nc.compile()
    _BUILD_CACHE[key] = nc
    return nc


# ---------------------------------------------------------------- host side
def _topk_idx(x, n):
    """Replicate reference token selection exactly (CPU jax; numpy fallback)."""
    try:
        import jax
        import jax.numpy as jnp
        cpu = jax.devices("cpu")[0]
        with jax.default_device(cpu):
            xj = jax.device_put(np.asarray(x), cpu)
            mean = jnp.mean(xj, axis=1, keepdims=True)
            mse = jnp.sum((xj - mean) ** 2, axis=-1)
            idx = jnp.argsort(-mse, axis=1)[:, :n]
            return np.asarray(idx)
    except Exception:
        x = np.asarray(x, np.float32)
        mean = x.mean(1, keepdims=True, dtype=np.float32)
        mse = ((x - mean) ** 2).sum(-1, dtype=np.float32)
        return np.argsort(-mse, axis=1, kind="stable")[:, :n]


# de-interleave permutation: [0,2,4,...,62, 1,3,...,63]
_DEINT = np.concatenate([np.arange(0, DH, 2), np.arange(1, DH, 2)])


def make_in_maps(x, cached_x, W_qkv, q_bias, v_bias, W_proj, b_proj,
                 scale_mul_log, rope_grid, idx, cfg):
    x = np.asarray(x, np.float32)
    W_qkv = np.asarray(W_qkv, np.float32)
    W_proj = np.asarray(W_proj, np.float32)
    q_bias = np.asarray(q_bias, np.float32)
    v_bias = np.asarray(v_bias, np.float32)
    rope_grid = np.asarray(rope_grid, np.float32)
    scale = np.exp(np.minimum(np.asarray(scale_mul_log, np.float32),
                              math.log(100.0))).reshape(NH)

    crow = cfg.C + 1 if cfg.has_bias else cfg.C
    n_groups = NH // cfg.HPG
    in_maps = []
    per_batch = {}
    for b in range(B):
        xm = x[b, idx[b]]                                   # (NREM, C)
        xmT = np.zeros((crow, cfg.NT), np.float32)
        xmT[:cfg.C, :cfg.NTR] = xm.T
        if cfg.has_bias:
            xmT[cfg.C, :cfg.NTR] = 1.0
        rc = rope_grid[0][idx[b]]                            # (NREM, DH//2)
        rs = rope_grid[1][idx[b]]
        ropeC = np.zeros((cfg.NT, cfg.DH), np.float32)
        ropeS = np.zeros((cfg.NT, cfg.DH), np.float32)
        ropeC[:cfg.NTR] = np.concatenate([rc, rc], axis=1)   # [c|c]
        ropeS[:cfg.NTR] = np.concatenate([rs, rs], axis=1)   # [s|s]
        per_batch[b] = (xmT, ropeC, ropeS)

    import ml_dtypes
    bf = ml_dtypes.bfloat16
    for core in range(N_CORES):
        b, hg = divmod(core, n_groups)
        hs = list(range(hg * cfg.HPG, (hg + 1) * cfg.HPG))
        xmT, ropeC, ropeS = per_batch[b]

        wq = np.zeros((crow, cfg.F), np.float32)
        HCb = cfg.HPG * cfg.DH
        for j, h in enumerate(hs):
            qrows = h * DH + _DEINT                  # de-interleaved q rows
            krows = C + h * DH + _DEINT              # de-interleaved k rows
            vrows = np.arange(2 * C + h * DH, 2 * C + (h + 1) * DH)
            wq[:cfg.C, j * DH:(j + 1) * DH] = W_qkv[qrows, :].T
            wq[:cfg.C, HCb + j * DH:HCb + (j + 1) * DH] = W_qkv[krows, :].T
            wq[:cfg.C, 2 * HCb + j * DH:2 * HCb + (j + 1) * DH] = W_qkv[vrows, :].T
            if cfg.has_bias:
                wq[cfg.C, j * DH:(j + 1) * DH] = q_bias[h * DH + _DEINT]
                wq[cfg.C, 2 * HCb + j * DH:2 * HCb + (j + 1) * DH] = \
                    v_bias[h * DH:(h + 1) * DH]

        cols = np.concatenate([np.arange(h * DH, (h + 1) * DH) for h in hs])
        wpT = W_proj[:, cols].T.copy()                      # (HC, C)

        scvec = np.concatenate([scale[hs], np.ones(cfg.HPG, np.float32)])

        in_maps.append({
            "ident": np.eye(128, dtype=np.float32).astype(bf),
            "xmT": xmT.astype(bf),
            "wqkvT": wq.astype(bf),
            "ropeC": ropeC.astype(bf),
            "ropeS": ropeS.astype(bf),
            "wpT": wpT.astype(bf),
            "scvec": scvec.reshape(1, 2 * cfg.HPG).astype(np.float32),
        })
    return in_maps


def kernel(x, cached_x, W_qkv, q_bias, v_bias, W_proj, b_proj,
           scale_mul_log, rope_grid, num_remain):
    n = int(num_remain)
    assert n == NREM, f"kernel compiled for num_remain={NREM}, got {n}"
    x = np.asarray(x, np.float32)
    cached_x = np.asarray(cached_x, np.float32)
    b_proj = np.asarray(b_proj, np.float32)

    idx = _topk_idx(x, n)
    has_bias = bool(np.any(np.asarray(q_bias)) or np.any(np.asarray(v_bias)))
    cfg = Cfg(NT=NT, NTR=NREM, C=C, HPG=HPG, DH=DH, has_bias=has_bias)
    in_maps = make_in_maps(x, cached_x, W_qkv, q_bias, v_bias, W_proj, b_proj,
                           scale_mul_log, rope_grid, idx, cfg)
    nc = build_full_program(has_bias=has_bias)
    res = bass_utils.run_bass_kernel_spmd(
        nc, in_maps, core_ids=list(range(N_CORES)))
    outs = [np.asarray(r["outp"], np.float32) for r in res.results]

    n_groups = NH // cfg.HPG
    o_full = np.zeros((B, n, C), np.float32)
    for b in range(B):
        acc = outs[b * n_groups][:n]
        for g in range(1, n_groups):
            acc = acc + outs[b * n_groups + g][:n]
        o_full[b] = acc + b_proj

    up = np.broadcast_to(
        cached_x[:, :, None, :, None, :], (B, 32, 2, 32, 2, C)
    ).reshape(B, L, C)
    out = x + up
    bix = np.arange(B)[:, None]
    out[bix, idx] = x[bix, idx] + o_full
    return out.astype(np.float32)
